# revision 10
# baseline (speedup 1.0000x reference)
"""Trainium2 Bass kernel for nn_FPSWE_pool (segment_reduce / sliced-Wasserstein pool).

Math (validated against the reference to ~4e-7 rel):
    W  = theta_v / ||theta_v||_row                       # [Pd, d_in]
    Xs = X @ W.T                                         # [N, Pd]
    S[e, :, p] = sort(Xs[e*32:(e+1)*32, p])              # per-edge, per-col sort
    out[e, p]  = c[p] - (1/M) * sum_r A[r, p] * S[e, r, p]
where A [32, Pd] and c [Pd] are small matrices computed on the host from
(weight, anchors, deg) only: A folds the anchor-grid linear interpolation,
the per-column argsort of anchors, and the weight matrix; c is the
edge-independent term (weight * anchors.T).mean(axis=1).

Sharding: edges are split 8 ways (contiguous 32-row degree blocks), per the
contiguous-block structure of hyperedge_index; params replicated.

Device work per core (256 edges = 8192 rows/core):
  1. one big DMA: [consts | X.T slice] fp32 into SBUF (4 range-chunks so
     compute can start early)
  2. fp32 matmul: Xs.T[ph, rows] = W_h @ X.T  (2 proj halves of 128)
  3. cast to bf16 (ScalarE copy), then a flip-form bitonic sort network
     (15 stages) over each 32-row block along the free dim on VectorE.
     A rotate-by-1 physical slot layout keeps every stage's innermost AP
     step at +-1 so the DVE bf16 2x perf mode applies.
  4. A-weighted reduce (tensor_tensor mult + tensor_reduce) + affine -> out.T
"""

import os
from contextlib import ExitStack

import numpy as np

E_EDGES, DEG, D_IN, N_PROJ, M_ANCH = 2048, 32, 128, 256, 128
N_CORES = 8
E_LOC = E_EDGES // N_CORES          # 256 edges per core
ROWS_LOC = E_LOC * DEG              # 8192 rows per core
PH = N_PROJ // 128                  # 2 projection halves
CONST_W = N_PROJ + 32 + PH          # wt(256) | A bf16-packed-as-f32(32) | c(2)
IN_W = CONST_W + ROWS_LOC           # full per-core input width (f32 cols)

LAST_RESULTS = None                 # test.py reads trace info from here


# ----------------------------------------------------------------- network ---
def _rot(i):
    return ((i << 1) | (i >> 4)) & 31


def _sort_stages(E):
    """Flip-form bitonic(32) stages in rotate-1 physical layout.

    Returns [(lo_off, lo_dims, hi_off, hi_dims)] over a free axis of E*32
    elements; each side covers E*16 elements, pairing elementwise in stream
    order; ascending logical order (min -> lo side).
    """
    out = []
    for m in range(1, 6):
        if m <= 4:
            lo_dims = [(1 << (m + 1), E * (1 << (4 - m)))]
            hi_dims = [(1 << (m + 1), E * (1 << (4 - m)))]
            if m >= 2:
                lo_dims.append((2, 1 << (m - 1)))
                hi_dims.append((-2, 1 << (m - 1)))
            lo_dims.append((1, 2))
            hi_dims.append((1, 2))
            out.append((0, lo_dims, (1 << (m + 1)) - 2, hi_dims))
        else:
            out.append((0, [(32, E), (2, 16)], 31, [(32, E), (-2, 16)]))
        d = (1 << m) // 4
        while d >= 1:
            f = d.bit_length()          # phys bit = k+1
            lo_dims = [(1 << (f + 1), E * (1 << (4 - f))), (1, 1 << f)]
            hi_dims = [(1 << (f + 1), E * (1 << (4 - f))), (1, 1 << f)]
            out.append((0, lo_dims, 1 << f, hi_dims))
            d //= 2
    assert len(out) == 15
    return out


# ------------------------------------------------------------- bass program ---
def _emit(tc, in_d, o_d):
    """Emit the per-core program.

    in_d [128, IN_W] f32: [ wt.T | A(bf16 packed) | c | X.T row-slice ]
    o_d  [PH, 128, E_LOC] f32: out.T per proj half
    """
    import concourse.mybir as mybir
    from concourse.ap import AP

    nc = tc.nc
    f32 = mybir.dt.float32
    bf16 = mybir.dt.bfloat16
    CH = 512                     # row-chunk per matmul
    NCH = ROWS_LOC // CH         # 16
    NDMA = 4                     # input DMA range-chunks

    with ExitStack() as ctx:
        main_p = ctx.enter_context(tc.tile_pool(name="main", bufs=1))
        ps_mm = ctx.enter_context(tc.tile_pool(name="ps_mm", bufs=2, space="PSUM"))
        sort_p = ctx.enter_context(tc.tile_pool(name="sort", bufs=1))
        out_p = ctx.enter_context(tc.tile_pool(name="out", bufs=2))

        xin = main_p.tile([128, IN_W], f32)
        step = ROWS_LOC // NDMA
        nc.sync.dma_start(xin[:, :CONST_W + step], in_d[:, :CONST_W + step])
        for g in range(1, NDMA):
            lo = CONST_W + g * step
            nc.sync.dma_start(xin[:, lo:lo + step], in_d[:, lo:lo + step])

        wt_sb = xin[:, :N_PROJ]
        a_sb = xin[:, N_PROJ:N_PROJ + 32].bitcast(bf16)      # [128, 64]
        c_sb = xin[:, N_PROJ + 32:N_PROJ + 32 + PH]
        xt = xin[:, CONST_W:]                                # [128, ROWS_LOC]

        # B0/B1 hold Xs.T bf16 per proj half; C0/C1 are sort ping-pong scratch.
        B = [sort_p.tile([128, ROWS_LOC], bf16, tag=f"B{h}", name=f"B{h}")
             for h in range(PH)]
        C = [sort_p.tile([128, ROWS_LOC], bf16, tag=f"C{h}", name=f"C{h}")
             for h in range(PH)]

        # Walrus allows only ONE sync-wait command on a Matmult (LDW struct).
        # Matmuls at DMA-group seams would need two (new DMA range + PSUM
        # slot release), so a tiny "gate" matmul absorbs each group's DMA
        # wait first; the real matmuls then only wait on the ACT copy.
        from concourse.bass import _add_dep_helper
        ps_gate = ctx.enter_context(tc.tile_pool(name="ps_gate", bufs=1, space="PSUM"))
        gates = {}
        for g in range(1, NDMA):
            pg = ps_gate.tile([128, 1], f32, tag=f"g{g}", name=f"gate{g}")
            gates[g] = nc.tensor.matmul(
                pg[:], wt_sb[:, :128],
                xt[:, g * (ROWS_LOC // NDMA):g * (ROWS_LOC // NDMA) + 1],
                start=True, stop=True,
            )

        # Same trick for the DVE: its first consts-range read (the A-broadcast
        # mult) must not carry both a DMA wait and a self wait (TT struct also
        # caps at one sync-wait command).
        dve_gate_t = out_p.tile([128, 2], f32, name="dve_gate_t")
        dve_gate = nc.vector.tensor_copy(dve_gate_t[:], c_sb[:, :PH])

        grp = NCH // NDMA
        for j in range(NCH):
            for h in range(PH):
                pmm = ps_mm.tile([128, CH], f32, tag=f"mm{h}", name=f"mm{h}_{j}")
                mm = nc.tensor.matmul(
                    pmm[:], wt_sb[:, h * 128:(h + 1) * 128],
                    xt[:, j * CH:(j + 1) * CH],
                    start=True, stop=True,
                )
                g = j // grp
                if g >= 1:
                    _add_dep_helper(
                        mm.ins, gates[g].ins, sync=False,
                        reason="order mm after its DMA-group gate",
                    )
                nc.scalar.copy(B[h][:, j * CH:(j + 1) * CH], pmm[:])

        stages = _sort_stages(E_LOC)
        alu_min = mybir.AluOpType.min
        alu_max = mybir.AluOpType.max

        def side_ap(tile, off, dims):
            base = tile[:]
            return AP(
                tensor=base.tensor,
                offset=base.offset + off,
                ap=[list(base.ap[0])] + [[s, c] for (s, c) in dims],
            )

        for h in range(PH):
            cur, oth = B[h], C[h]
            first_tt = True
            for (lo_off, lo_dims, hi_off, hi_dims) in stages:
                for op, w_off, w_dims in (
                    (alu_min, lo_off, lo_dims),
                    (alu_max, hi_off, hi_dims),
                ):
                    tt = nc.vector.tensor_tensor(
                        out=side_ap(oth, w_off, w_dims),
                        in0=side_ap(cur, lo_off, lo_dims),
                        in1=side_ap(cur, hi_off, hi_dims),
                        op=op,
                    )
                    if first_tt:
                        _add_dep_helper(
                            tt.ins, dve_gate.ins, sync=False,
                            reason="order sort after DVE consts gate",
                        )
                        first_tt = False
                cur, oth = oth, cur

            # cur holds the sorted data; oth is free scratch.
            a_h = a_sb[:, h * DEG:(h + 1) * DEG].unsqueeze(1).broadcast_to(
                [128, E_LOC, DEG]
            )
            prod = oth
            nc.vector.tensor_tensor(
                out=prod[:].rearrange("p (e d) -> p e d", d=DEG),
                in0=cur[:].rearrange("p (e d) -> p e d", d=DEG),
                in1=a_h,
                op=mybir.AluOpType.mult,
            )
            osb = out_p.tile([128, E_LOC], f32)
            nc.vector.tensor_reduce(
                out=osb[:],
                in_=prod[:].rearrange("p (e d) -> p e d", d=DEG),
                axis=mybir.AxisListType.X,
                op=mybir.AluOpType.add,
            )
            nc.vector.tensor_scalar(
                out=osb[:], in0=osb[:],
                scalar1=-1.0 / M_ANCH, scalar2=c_sb[:, h:h + 1],
                op0=mybir.AluOpType.mult, op1=mybir.AluOpType.add,
            )
            nc.sync.dma_start(o_d[h], osb[:])


def _build():
    import concourse.bacc as bacc
    import concourse.mybir as mybir
    import concourse.tile as tile

    nc = bacc.Bacc(
        "TRN2", target_bir_lowering=False, debug=False,
        enable_asserts=False, num_devices=N_CORES,
    )
    f32 = mybir.dt.float32
    in_d = nc.dram_tensor("xtc", [128, IN_W], f32, kind="ExternalInput").ap()
    o_d = nc.dram_tensor(
        "o", [PH, 128, E_LOC], f32, kind="ExternalOutput"
    ).ap()
    with tile.TileContext(nc) as tc:
        _emit(tc, in_d, o_d)
    nc.compile()
    return nc


_CACHE = {}


def _host_consts(theta_v, weight, anchors):
    import ml_dtypes

    W = theta_v / np.linalg.norm(theta_v, axis=1, keepdims=True)
    u = np.linspace(0.0, 1.0, M_ANCH, dtype=np.float32) * np.float32(0.99998)
    a = u * np.float32(DEG - 1.0) / np.float32(0.99999)
    r0 = np.clip(np.floor(a), 0.0, DEG - 2.0)
    frac = (a - r0).astype(np.float32)
    r0 = r0.astype(np.int64)
    Rind = np.argsort(anchors, axis=0, kind="stable")          # [M, Pd]
    wperm = np.zeros((M_ANCH, N_PROJ), np.float32)
    np.put_along_axis(wperm, Rind, weight.T, axis=0)
    A = np.zeros((DEG, N_PROJ), np.float32)
    np.add.at(A, r0, wperm * (1.0 - frac)[:, None])
    np.add.at(A, r0 + 1, wperm * frac[:, None])
    c = (weight * anchors.T).mean(axis=1).astype(np.float32)   # [Pd]

    # physical slot layout: rank r lives at slot rot(r)
    A_phys = np.zeros_like(A)
    for r in range(DEG):
        A_phys[_rot(r)] = A[r]
    A2 = np.zeros((128, PH * DEG), np.float32)
    c2 = np.zeros((128, PH), np.float32)
    for h in range(PH):
        A2[:, h * DEG:(h + 1) * DEG] = A_phys[:, h * 128:(h + 1) * 128].T
        c2[:, h] = c[h * 128:(h + 1) * 128]
    A2_packed = (
        A2.astype(ml_dtypes.bfloat16).view(np.uint16)
        .reshape(128, PH * DEG).view(np.uint32).view(np.float32)
    )                                                          # [128, 32]
    consts = np.zeros((128, CONST_W), np.float32)
    consts[:, :N_PROJ] = np.ascontiguousarray(W.T, dtype=np.float32)
    consts[:, N_PROJ:N_PROJ + 32] = A2_packed
    consts[:, N_PROJ + 32:N_PROJ + 32 + PH] = c2
    return consts


def kernel(X, hyperedge_index, theta_v, weight, anchors, num_edges):
    global LAST_RESULTS
    from concourse.bass_utils import run_bass_kernel_spmd

    X = np.asarray(X, dtype=np.float32)
    theta_v = np.asarray(theta_v, dtype=np.float32)
    weight = np.asarray(weight, dtype=np.float32)
    anchors = np.asarray(anchors, dtype=np.float32)

    consts = _host_consts(theta_v, weight, anchors)
    XT = np.ascontiguousarray(X.T)                             # [128, N]
    if "nc" not in _CACHE:
        _CACHE["nc"] = _build()
    nc = _CACHE["nc"]

    in_maps = []
    for cid in range(N_CORES):
        xtc = np.empty((128, IN_W), np.float32)
        xtc[:, :CONST_W] = consts
        xtc[:, CONST_W:] = XT[:, cid * ROWS_LOC:(cid + 1) * ROWS_LOC]
        in_maps.append({"xtc": xtc})
    res = run_bass_kernel_spmd(
        nc, in_maps, core_ids=list(range(N_CORES)),
        trace=bool(int(os.environ.get("KERNEL_TRACE", "0"))),
    )
    LAST_RESULTS = res

    outT = np.empty((N_PROJ, E_EDGES), np.float32)
    for cid in range(N_CORES):
        o = res.results[cid]["o"]                    # [PH, 128, E_LOC]
        outT[:, cid * E_LOC:(cid + 1) * E_LOC] = o.reshape(N_PROJ, E_LOC)
    return np.ascontiguousarray(outT.T)


# revision 13
# speedup vs baseline: 1.0835x; 1.0835x over previous
"""Trainium2 Bass kernel for nn_FPSWE_pool (segment_reduce / sliced-Wasserstein pool).

Math (validated against the reference to ~4e-7 rel):
    W  = theta_v / ||theta_v||_row                       # [Pd, d_in]
    Xs = X @ W.T                                         # [N, Pd]
    S[e, :, p] = sort(Xs[e*32:(e+1)*32, p])              # per-edge, per-col sort
    out[e, p]  = c[p] - (1/M) * sum_r A[r, p] * S[e, r, p]
where A [32, Pd] and c [Pd] are small matrices computed on the host from
(weight, anchors, deg) only: A folds the anchor-grid linear interpolation,
the per-column argsort of anchors, and the weight matrix; c is the
edge-independent term (weight * anchors.T).mean(axis=1).

Sharding: edges are split 8 ways (contiguous 32-row degree blocks), per the
contiguous-block structure of hyperedge_index; params replicated.

Device work per core (256 edges = 8192 rows/core):
  1. one big DMA: [consts | X.T slice] fp32 into SBUF (4 range-chunks so
     compute can start early)
  2. fp32 matmul: Xs.T[ph, rows] = W_h @ X.T  (2 proj halves of 128)
  3. cast to bf16 (ScalarE copy), then a flip-form bitonic sort network
     (15 stages) over each 32-row block along the free dim on VectorE.
     A rotate-by-1 physical slot layout keeps every stage's innermost AP
     step at +-1 so the DVE bf16 2x perf mode applies.
  4. A-weighted reduce (tensor_tensor mult + tensor_reduce) + affine -> out.T
"""

import os
from contextlib import ExitStack

import numpy as np

E_EDGES, DEG, D_IN, N_PROJ, M_ANCH = 2048, 32, 128, 256, 128
N_CORES = 8
E_LOC = E_EDGES // N_CORES          # 256 edges per core
ROWS_LOC = E_LOC * DEG              # 8192 rows per core
PH = N_PROJ // 128                  # 2 projection halves
CONST_W = N_PROJ + 32 + PH          # wt(256) | A bf16-packed-as-f32(32) | c(2)
IN_W = CONST_W + ROWS_LOC           # full per-core input width (f32 cols)

LAST_RESULTS = None                 # test.py reads trace info from here


# ----------------------------------------------------------------- network ---
def _rot(i):
    return ((i << 1) | (i >> 4)) & 31


def _sort_stages(E):
    """Flip-form bitonic(32) stages in rotate-1 physical layout.

    Returns [(lo_off, lo_dims, hi_off, hi_dims)] over a free axis of E*32
    elements; each side covers E*16 elements, pairing elementwise in stream
    order; ascending logical order (min -> lo side).
    """
    out = []
    for m in range(1, 6):
        if m <= 4:
            lo_dims = [(1 << (m + 1), E * (1 << (4 - m)))]
            hi_dims = [(1 << (m + 1), E * (1 << (4 - m)))]
            if m >= 2:
                lo_dims.append((2, 1 << (m - 1)))
                hi_dims.append((-2, 1 << (m - 1)))
            lo_dims.append((1, 2))
            hi_dims.append((1, 2))
            out.append((0, lo_dims, (1 << (m + 1)) - 2, hi_dims))
        else:
            out.append((0, [(32, E), (2, 16)], 31, [(32, E), (-2, 16)]))
        d = (1 << m) // 4
        while d >= 1:
            f = d.bit_length()          # phys bit = k+1
            lo_dims = [(1 << (f + 1), E * (1 << (4 - f))), (1, 1 << f)]
            hi_dims = [(1 << (f + 1), E * (1 << (4 - f))), (1, 1 << f)]
            out.append((0, lo_dims, 1 << f, hi_dims))
            d //= 2
    assert len(out) == 15
    return out


# ------------------------------------------------------------- bass program ---
def _emit(tc, in_d, o_d):
    """Emit the per-core program.

    in_d [128, IN_W] f32: [ wt.T | A(bf16 packed) | c | X.T row-slice ]
    o_d  [PH, 128, E_LOC] f32: out.T per proj half
    """
    import concourse.mybir as mybir
    from concourse.ap import AP

    nc = tc.nc
    f32 = mybir.dt.float32
    bf16 = mybir.dt.bfloat16
    CH = 512                     # row-chunk per matmul
    NCH = ROWS_LOC // CH         # 16
    NDMA = 4                     # input DMA range-chunks

    with ExitStack() as ctx:
        main_p = ctx.enter_context(tc.tile_pool(name="main", bufs=1))
        ps_mm = ctx.enter_context(tc.tile_pool(name="ps_mm", bufs=2, space="PSUM"))
        sort_p = ctx.enter_context(tc.tile_pool(name="sort", bufs=1))
        out_p = ctx.enter_context(tc.tile_pool(name="out", bufs=2))

        xin = main_p.tile([128, IN_W], f32)
        step = ROWS_LOC // NDMA
        nc.sync.dma_start(xin[:, :CONST_W + step], in_d[:, :CONST_W + step])
        for g in range(1, NDMA):
            lo = CONST_W + g * step
            nc.sync.dma_start(xin[:, lo:lo + step], in_d[:, lo:lo + step])

        wt_sb = xin[:, :N_PROJ]
        a_sb = xin[:, N_PROJ:N_PROJ + 32].bitcast(bf16)      # [128, 64]
        c_sb = xin[:, N_PROJ + 32:N_PROJ + 32 + PH]
        xt = xin[:, CONST_W:]                                # [128, ROWS_LOC]

        # B0/B1 hold Xs.T bf16 per proj half; C0/C1 are sort ping-pong scratch.
        B = [sort_p.tile([128, ROWS_LOC], bf16, tag=f"B{h}", name=f"B{h}")
             for h in range(PH)]
        C = [sort_p.tile([128, ROWS_LOC], bf16, tag=f"C{h}", name=f"C{h}")
             for h in range(PH)]

        # Walrus allows only ONE sync-wait command on a Matmult (LDW struct).
        # Matmuls at DMA-group seams would need two (new DMA range + PSUM
        # slot release), so a tiny "gate" matmul absorbs each group's DMA
        # wait first; the real matmuls then only wait on the ACT copy.
        from concourse.bass import _add_dep_helper
        ps_gate = ctx.enter_context(tc.tile_pool(name="ps_gate", bufs=1, space="PSUM"))
        gates = {}
        for g in range(1, NDMA):
            pg = ps_gate.tile([128, 1], f32, tag=f"g{g}", name=f"gate{g}")
            gates[g] = nc.tensor.matmul(
                pg[:], wt_sb[:, :128],
                xt[:, g * (ROWS_LOC // NDMA):g * (ROWS_LOC // NDMA) + 1],
                start=True, stop=True,
            )

        # Same trick for the DVE: its first consts-range read (the A-broadcast
        # mult) must not carry both a DMA wait and a self wait (TT struct also
        # caps at one sync-wait command).
        dve_gate_t = out_p.tile([128, 2], f32, name="dve_gate_t")
        dve_gate = nc.vector.tensor_copy(dve_gate_t[:], c_sb[:, :PH])

        grp = NCH // NDMA
        for h in range(PH):
            for j in range(NCH):
                pmm = ps_mm.tile([128, CH], f32, tag=f"mm{h}", name=f"mm{h}_{j}")
                mm = nc.tensor.matmul(
                    pmm[:], wt_sb[:, h * 128:(h + 1) * 128],
                    xt[:, j * CH:(j + 1) * CH],
                    start=True, stop=True,
                )
                g = j // grp
                if g >= 1:
                    _add_dep_helper(
                        mm.ins, gates[g].ins, sync=False,
                        reason="order mm after its DMA-group gate",
                    )
                nc.scalar.copy(B[h][:, j * CH:(j + 1) * CH], pmm[:])

        EC = 2                          # edge-chunks per proj half
        ECE = E_LOC // EC               # 128 edges per chunk
        ecols = ECE * DEG               # 4096 free columns per chunk
        stages = _sort_stages(ECE)
        alu_min = mybir.AluOpType.min
        alu_max = mybir.AluOpType.max

        def side_ap(tile, off, dims):
            base = tile[:]
            return AP(
                tensor=base.tensor,
                offset=base.offset + off,
                ap=[list(base.ap[0])] + [[s, c] for (s, c) in dims],
            )

        otile = out_p.tile([128, PH * E_LOC], f32, name="otile")

        for h in range(PH):
            for cch in range(EC):
                co = cch * ecols
                cur, oth = B[h], C[h]
                first_tt = (h == 0 and cch == 0)
                for (lo_off, lo_dims, hi_off, hi_dims) in stages:
                    for op, w_off, w_dims in (
                        (alu_min, lo_off, lo_dims),
                        (alu_max, hi_off, hi_dims),
                    ):
                        tt = nc.vector.tensor_tensor(
                            out=side_ap(oth, co + w_off, w_dims),
                            in0=side_ap(cur, co + lo_off, lo_dims),
                            in1=side_ap(cur, co + hi_off, hi_dims),
                            op=op,
                        )
                        if first_tt:
                            _add_dep_helper(
                                tt.ins, dve_gate.ins, sync=False,
                                reason="order sort after DVE consts gate",
                            )
                            first_tt = False
                    cur, oth = oth, cur

                # cur holds the sorted chunk; oth is scratch.
                a_h = a_sb[:, h * DEG:(h + 1) * DEG].unsqueeze(1).broadcast_to(
                    [128, ECE, DEG]
                )
                nc.vector.tensor_tensor(
                    out=side_ap(oth, co, [(DEG, ECE), (1, DEG)]),
                    in0=side_ap(cur, co, [(DEG, ECE), (1, DEG)]),
                    in1=a_h,
                    op=mybir.AluOpType.mult,
                )
                # level-1 pairwise add (bf16, in place), then fp32 reduce of 16
                lo16 = side_ap(oth, co, [(DEG, ECE), (1, 16)])
                nc.vector.tensor_tensor(
                    out=lo16, in0=lo16,
                    in1=side_ap(oth, co + 16, [(DEG, ECE), (1, 16)]),
                    op=mybir.AluOpType.add,
                )
                osl = otile[:, h * E_LOC + cch * ECE:h * E_LOC + (cch + 1) * ECE]
                nc.vector.tensor_reduce(
                    out=osl, in_=lo16,
                    axis=mybir.AxisListType.X, op=mybir.AluOpType.add,
                )
                nc.vector.tensor_scalar(
                    out=osl, in0=osl,
                    scalar1=-1.0 / M_ANCH, scalar2=c_sb[:, h:h + 1],
                    op0=mybir.AluOpType.mult, op1=mybir.AluOpType.add,
                )

        for h in range(PH):
            nc.sync.dma_start(o_d[h], otile[:, h * E_LOC:(h + 1) * E_LOC])


def _build():
    import concourse.bacc as bacc
    import concourse.mybir as mybir
    import concourse.tile as tile

    nc = bacc.Bacc(
        "TRN2", target_bir_lowering=False, debug=False,
        enable_asserts=False, num_devices=N_CORES,
    )
    f32 = mybir.dt.float32
    in_d = nc.dram_tensor("xtc", [128, IN_W], f32, kind="ExternalInput").ap()
    o_d = nc.dram_tensor(
        "o", [PH, 128, E_LOC], f32, kind="ExternalOutput"
    ).ap()
    with tile.TileContext(nc) as tc:
        _emit(tc, in_d, o_d)
    nc.compile()
    return nc


_CACHE = {}


def _host_consts(theta_v, weight, anchors):
    import ml_dtypes

    W = theta_v / np.linalg.norm(theta_v, axis=1, keepdims=True)
    u = np.linspace(0.0, 1.0, M_ANCH, dtype=np.float32) * np.float32(0.99998)
    a = u * np.float32(DEG - 1.0) / np.float32(0.99999)
    r0 = np.clip(np.floor(a), 0.0, DEG - 2.0)
    frac = (a - r0).astype(np.float32)
    r0 = r0.astype(np.int64)
    Rind = np.argsort(anchors, axis=0, kind="stable")          # [M, Pd]
    wperm = np.zeros((M_ANCH, N_PROJ), np.float32)
    np.put_along_axis(wperm, Rind, weight.T, axis=0)
    A = np.zeros((DEG, N_PROJ), np.float32)
    np.add.at(A, r0, wperm * (1.0 - frac)[:, None])
    np.add.at(A, r0 + 1, wperm * frac[:, None])
    c = (weight * anchors.T).mean(axis=1).astype(np.float32)   # [Pd]

    # physical slot layout: rank r lives at slot rot(r)
    A_phys = np.zeros_like(A)
    for r in range(DEG):
        A_phys[_rot(r)] = A[r]
    A2 = np.zeros((128, PH * DEG), np.float32)
    c2 = np.zeros((128, PH), np.float32)
    for h in range(PH):
        A2[:, h * DEG:(h + 1) * DEG] = A_phys[:, h * 128:(h + 1) * 128].T
        c2[:, h] = c[h * 128:(h + 1) * 128]
    A2_packed = (
        A2.astype(ml_dtypes.bfloat16).view(np.uint16)
        .reshape(128, PH * DEG).view(np.uint32).view(np.float32)
    )                                                          # [128, 32]
    consts = np.zeros((128, CONST_W), np.float32)
    consts[:, :N_PROJ] = np.ascontiguousarray(W.T, dtype=np.float32)
    consts[:, N_PROJ:N_PROJ + 32] = A2_packed
    consts[:, N_PROJ + 32:N_PROJ + 32 + PH] = c2
    return consts


def kernel(X, hyperedge_index, theta_v, weight, anchors, num_edges):
    global LAST_RESULTS
    from concourse.bass_utils import run_bass_kernel_spmd

    X = np.asarray(X, dtype=np.float32)
    theta_v = np.asarray(theta_v, dtype=np.float32)
    weight = np.asarray(weight, dtype=np.float32)
    anchors = np.asarray(anchors, dtype=np.float32)

    consts = _host_consts(theta_v, weight, anchors)
    XT = np.ascontiguousarray(X.T)                             # [128, N]
    if "nc" not in _CACHE:
        _CACHE["nc"] = _build()
    nc = _CACHE["nc"]

    in_maps = []
    for cid in range(N_CORES):
        xtc = np.empty((128, IN_W), np.float32)
        xtc[:, :CONST_W] = consts
        xtc[:, CONST_W:] = XT[:, cid * ROWS_LOC:(cid + 1) * ROWS_LOC]
        in_maps.append({"xtc": xtc})
    res = run_bass_kernel_spmd(
        nc, in_maps, core_ids=list(range(N_CORES)),
        trace=bool(int(os.environ.get("KERNEL_TRACE", "0"))),
    )
    LAST_RESULTS = res

    outT = np.empty((N_PROJ, E_EDGES), np.float32)
    for cid in range(N_CORES):
        o = res.results[cid]["o"]                    # [PH, 128, E_LOC]
        outT[:, cid * E_LOC:(cid + 1) * E_LOC] = o.reshape(N_PROJ, E_LOC)
    return np.ascontiguousarray(outT.T)
